# revision 1
# baseline (speedup 1.0000x reference)
"""Trainium2 Bass kernel for AttentionL2 (B=4, S=4096, DIN=384, DOUT=64).

out = softmax(cdist(q, k) / 8, axis=-1) @ v  with q/k/v = x @ W{q,k,v}.T

Sharding: 8 cores = 4 batches x 2 query-halves. Each core receives the
full x of its batch, host pre-transposed to x^T (bf16 -- identical to
the on-chip cast the matmul needs anyway) with rows reordered so its own
query half comes first (softmax over keys is permutation invariant).
Every core runs the same SPMD program: q = columns 0:2048, keys = all.

Per-core math (matmuls bf16 with fp32 accumulation):
  d2[j,i] = |q_i - k_j|^2 via one augmented matmul with the contraction
  padded to 128 rows (zeros) so the PE's activity monitor unthrottles:
      lhsT = [-2*k^T; k2; 1; 0...]  (128 x 128 keys per tile)
      rhs  = [q^T; 1; q2; 0...]     (128 x 2048)
  att = exp(sqrt(d2)/8) (unnormalized; distances are O(10), no overflow,
  softmax needs no max subtraction), two engine paths tile-by-tile:
   - ScalarE: Sqrt(d2/64) -> fp16 buffer; after a scheduler barrier (one
     ACT table switch) Exp with bias -2*ln(c0) -> bf16
   - VectorE: one custom DVE op (p(z)/c0)^2, p = minimax cubic of
     exp(sqrt(z)/16): the whole exp(sqrt(z)/8)/c0^2 in a single pass
  outT = [v; 1; 0...]^T @ att  (row 64 = softmax denominator, PSUM f32)
Final normalize outT[0:64]/outT[64] + transpose happen on the host.
"""

from contextlib import ExitStack

import ml_dtypes
import numpy as np

import concourse.bacc as bacc
import concourse.mybir as mybir
import concourse.tile as tile
from concourse import dve_ops
from concourse.dve_spec import Spec, Src0, C0, C1, C2, One, lower
from concourse.dve_uop import DveOpSpec
from concourse.bass_utils import run_bass_kernel_spmd

F32 = mybir.dt.float32
BF16 = mybir.dt.bfloat16
F16 = mybir.dt.float16
AF = mybir.ActivationFunctionType

B, S, DIN, DOUT = 4, 4096, 384, 64
M = S // 2        # query rows per core
KT = S // 128     # 32 key tiles
DC = DIN // 128   # 3 contraction chunks
NCORES = 8
MMN = 512         # matmul moving free dim (psum out must stay in one bank)

# minimax cubic p for exp(sqrt(z)/16) on z in [32, 312], normalized by its
# constant term so the Horner tail can use the hardware One constant
# (a [P,1]-broadcast Src1 crashes the DVE, so only 3 scalar slots exist).
# att_dve = (p(z)/c0)^2 = exp(sqrt(z)/8)/c0^2; the ACT path matches the
# 1/c0^2 scale via a constant bias in its Exp (softmax is scale-invariant).
PA = 1.6518381642404523e-08
PB = -1.037933864407201e-05
PC = 0.006602996452846391
EXP_BIAS = -0.3424032850267295  # -2*ln(c0)

# key tiles handled by the custom-DVE composite path (rest: ACT sqrt/exp)
N_DVE = 15


def _register_dve_op():
    name = "EXP_SQRT_SQ_ANT"
    if name in dve_ops._SUB_OPCODE_FOR_NAME:
        return next(op for op in dve_ops.OPS if op.name == name)
    t = ((Src0 * C0 + C1) * Src0 + C2) * Src0 + One
    body = t * t

    def ref(in0, in1, c0, c1, c2):
        tt = ((in0 * c0 + c1) * in0 + c2) * in0 + 1.0
        return tt * tt

    spec = Spec(body=body, reference=ref)
    row = max(dve_ops._SUB_OPCODE_FOR_NAME.values()) + 1
    assert row < 0x20
    dve_ops._SUB_OPCODE_FOR_NAME[name] = row
    shas = {}
    for ver in ("v3", "v4"):
        try:
            uops = lower(spec, ver=ver)
            shas[ver] = DveOpSpec(
                name=name, opcode=row, uops=uops, rd1_en=False
            ).sha(ver)
        except Exception:
            pass
    op = dve_ops.DveOp(name, spec, subdim=False, uops_sha=shas)
    dve_ops.OPS.append(op)
    dve_ops.CUSTOM_DVE_SPECS[name] = spec
    return op


EXP_OP = _register_dve_op()


def _is_dve_tile(n):
    # spread DVE tiles evenly among the 32 key tiles
    return (n * N_DVE) % KT >= KT - N_DVE


def _body(tc, xt, wt, out):
    nc = tc.nc
    assert sum(_is_dve_tile(n) for n in range(KT)) == N_DVE

    with ExitStack() as ctx:
        const_pool = ctx.enter_context(tc.tile_pool(name="const", bufs=1))
        ones64 = const_pool.tile([64, 1], BF16)
        nc.vector.memset(ones64[:], 1.0)
        ones64x2 = const_pool.tile([64, 2], BF16)
        nc.vector.memset(ones64x2[:], 1.0)
        ebias = const_pool.tile([128, 1], F32)
        nc.vector.memset(ebias[:], EXP_BIAS)

        main_pool = ctx.enter_context(tc.tile_pool(name="main", bufs=1))
        kT_aug = main_pool.tile([128, S], BF16)
        qT_aug = main_pool.tile([128, M], BF16)
        v_sb = main_pool.tile([128, KT, 128], BF16)

        # ---------------- setup: load x^T/W^T (bf16), project ----------------
        # Ordered so the q side (which every phase-1 matmul needs in full)
        # finishes first, the k side streams per-chunk, and the v transposes
        # ride both HWDGE rings underneath phase 1 (v is needed in phase 2).
        with ExitStack() as sctx:
            xp = sctx.enter_context(tc.tile_pool(name="xsb", bufs=1))
            xT = xp.tile([128, DC, S], BF16)
            wT = xp.tile([128, DC, 3 * DOUT], BF16)
            vT = xp.tile([64, S], BF16)
            tmp_sq = xp.tile([64, S], BF16, tag="sq")
            xt_r = xt.rearrange("(c p) s -> p c s", p=128)
            wt_r = wt.rearrange("(c p) w -> p c w", p=128)
            # dependency-free memsets first
            nc.vector.memset(kT_aug[64:128, :], 0.0)
            nc.vector.memset(kT_aug[64:66, :], 1.0)
            nc.vector.memset(qT_aug[64:128, :], 0.0)
            nc.gpsimd.memset(v_sb[:, :, 64:128], 0.0)
            for c in range(DC):
                nc.sync.dma_start(wT[:, c, :], wt_r[:, c, :])
                nc.sync.dma_start(xT[:, c, 0:M], xt_r[:, c, 0:M])
            for c in range(DC):
                nc.sync.dma_start(xT[:, c, M:S], xt_r[:, c, M:S])

            pp_pool = sctx.enter_context(
                tc.tile_pool(name="pp", bufs=3, space="PSUM")
            )

            # q side first: qT_aug rows 0:64 = q^T, row 64 = 1, row 65 = q2
            for ss in range(M // 512):
                sl = slice(ss * 512, (ss + 1) * 512)
                pq = pp_pool.tile([64, 512], F32, tag="p")
                for c in range(DC):
                    nc.tensor.matmul(
                        pq[:],
                        wT[:, c, 0:64],
                        xT[:, c, sl],
                        start=(c == 0),
                        stop=(c == DC - 1),
                    )
                nc.vector.tensor_copy(qT_aug[0:64, sl], pq[:])
                nc.vector.tensor_mul(
                    tmp_sq[:, sl], qT_aug[0:64, sl], qT_aug[0:64, sl]
                )
                p2q = pp_pool.tile([2, 512], F32, tag="p")
                nc.tensor.matmul(
                    p2q[:], ones64x2[:], tmp_sq[:, sl], start=True, stop=True
                )
                nc.vector.tensor_copy(qT_aug[64:66, sl], p2q[:])
            nc.vector.memset(qT_aug[64:65, :], 1.0)

            # k and v per 512-chunk: kT_aug rows = -2k^T / k2 / 1 / 0...,
            # vT = v^T (transposed to v_sb below)
            for ss in range(S // 512):
                sl = slice(ss * 512, (ss + 1) * 512)
                pk = pp_pool.tile([64, 512], F32, tag="p")
                for c in range(DC):
                    nc.tensor.matmul(
                        pk[:],
                        wT[:, c, 64:128],
                        xT[:, c, sl],
                        start=(c == 0),
                        stop=(c == DC - 1),
                    )
                nc.vector.tensor_scalar_mul(kT_aug[0:64, sl], pk[:], -2.0)
                nc.vector.tensor_mul(
                    tmp_sq[:, sl], kT_aug[0:64, sl], kT_aug[0:64, sl]
                )
                p2 = pp_pool.tile([1, 512], F32, tag="p")
                nc.tensor.matmul(
                    p2[:], ones64[:], tmp_sq[:, sl], start=True, stop=True
                )
                # rows held -2k so the sum is 4*k2
                nc.vector.tensor_scalar_mul(kT_aug[64:65, sl], p2[:], 0.25)

                pv = pp_pool.tile([64, 512], F32, tag="p")
                for c in range(DC):
                    nc.tensor.matmul(
                        pv[:],
                        wT[:, c, 128:192],
                        xT[:, c, sl],
                        start=(c == 0),
                        stop=(c == DC - 1),
                    )
                nc.vector.tensor_copy(vT[:, sl], pv[:])
                # v_sb[:, n, j] = v[128n+p, j] via SBUF->SBUF xbar DMA
                # transpose, alternating the two HWDGE rings
                for j in range(4):
                    n = ss * 4 + j
                    eng = nc.sync if n % 2 == 0 else nc.scalar
                    eng.dma_start_transpose(
                        v_sb[:, n, 0:64], vT[:, n * 128 : (n + 1) * 128]
                    )
            nc.gpsimd.memset(v_sb[:, :, 64:65], 1.0)

        # shared buffer: fp16 dist (ACT tiles) or bf16 att (DVE tiles)
        buf_pool = ctx.enter_context(tc.tile_pool(name="buf", bufs=1))
        buf = buf_pool.tile([128, KT, M], F16)

        # ---------------- phase 1: d2 matmul + sqrt/composite ----------------
        # ps tiles are half-width (2 banks) so this pool coexists with the
        # setup psum pool and phase 1 overlaps the tail of setup.
        with tc.tile_pool(name="ps", bufs=2, space="PSUM") as ps_pool:
            for n in range(KT):
                for h in range(2):
                    hsl = slice(h * (M // 2), (h + 1) * (M // 2))
                    ps = ps_pool.tile([128, M // 2], F32)
                    for ss in range(M // 2 // MMN):
                        nc.tensor.matmul(
                            ps[:, ss * MMN : (ss + 1) * MMN],
                            kT_aug[:, n * 128 : (n + 1) * 128],
                            qT_aug[:, h * (M // 2) + ss * MMN : h * (M // 2) + (ss + 1) * MMN],
                            start=True,
                            stop=True,
                        )
                    if _is_dve_tile(n):
                        # att/c0^2 = (p(d2)/c0)^2 in one pass, as bf16
                        nc.vector._custom_dve(
                            EXP_OP,
                            out=buf[:, n, hsl].bitcast(BF16),
                            in0=ps[:],
                            s0=PA,
                            s1=PB,
                            imm2=PC,
                        )
                    else:
                        # dist/8 = sqrt(d2/64), fp16
                        nc.scalar.activation(
                            buf[:, n, hsl], ps[:], AF.Sqrt, scale=1.0 / 64.0
                        )

        tc.no_sync_barrier()  # all Sqrt before all Exp: one table switch

        # ---------------- phase 2: exp (ACT tiles) + [v;1]^T @ att ----------------
        with ExitStack() as p2ctx:
            po_pool = p2ctx.enter_context(
                tc.tile_pool(name="po", bufs=1, space="PSUM")
            )
            att_pool = p2ctx.enter_context(tc.tile_pool(name="att", bufs=3))
            po = po_pool.tile([128, M], F32)
            for n in range(KT):
                if _is_dve_tile(n):
                    att_ap = buf[:, n, :].bitcast(BF16)
                else:
                    att = att_pool.tile([128, M], BF16)
                    nc.scalar.activation(
                        att[:], buf[:, n, :], AF.Exp, bias=ebias[:]
                    )
                    att_ap = att[:]
                for ss in range(M // MMN):
                    nc.tensor.matmul(
                        po[:, ss * MMN : (ss + 1) * MMN],
                        v_sb[:, n, :],
                        att_ap[:, ss * MMN : (ss + 1) * MMN],
                        start=(n == 0),
                        stop=(n == KT - 1),
                    )

            # -------- finish: copy outT[0:65] to SBUF, DMA out --------
            oT_pool = p2ctx.enter_context(tc.tile_pool(name="oT", bufs=1))
            oT = oT_pool.tile([65, M], F32)
            nc.vector.tensor_copy(oT[:], po[0:65, :])
            nc.sync.dma_start(out[:, :], oT[:])


_NC_CACHE = None


def build():
    global _NC_CACHE
    if _NC_CACHE is not None:
        return _NC_CACHE
    nc = bacc.Bacc("TRN2", target_bir_lowering=False, debug=False, num_devices=NCORES)
    xt_d = nc.declare_dram_parameter("xt", [DIN, S], BF16, isOutput=False)
    wt_d = nc.declare_dram_parameter("wt", [DIN, 3 * DOUT], BF16, isOutput=False)
    out_d = nc.declare_dram_parameter("out", [65, M], F32, isOutput=True)
    with tile.TileContext(nc) as tc:
        _body(tc, xt_d[:], wt_d[:], out_d[:])
    nc.compile()
    _NC_CACHE = nc
    return nc


def make_in_maps(x, Wq, Wk, Wv):
    bf16 = ml_dtypes.bfloat16
    wt = np.ascontiguousarray(
        np.concatenate(
            [np.asarray(W, np.float32).T for W in (Wq, Wk, Wv)], axis=1
        ).astype(bf16)
    )
    in_maps = []
    for c in range(NCORES):
        b, h = divmod(c, 2)
        xb = np.asarray(x[b], np.float32)
        xc = np.concatenate(
            [xb[h * M : (h + 1) * M], xb[(1 - h) * M : (2 - h) * M]], 0
        )
        in_maps.append({"xt": np.ascontiguousarray(xc.T.astype(bf16)), "wt": wt})
    return in_maps


def gather_out(results):
    out = np.zeros((B, S, DOUT), np.float32)
    for c in range(NCORES):
        b, h = divmod(c, 2)
        oT = np.asarray(results[c]["out"], np.float32)
        out[b, h * M : (h + 1) * M] = (oT[0:64] / oT[64:65]).T
    return out


def kernel(x, Wq, Wk, Wv):
    nc = build()
    in_maps = make_in_maps(x, Wq, Wk, Wv)
    res = run_bass_kernel_spmd(nc, in_maps, core_ids=list(range(NCORES)))
    return gather_out(res.results)



# revision 15
# speedup vs baseline: 1.0571x; 1.0571x over previous
"""Trainium2 Bass kernel for AttentionL2 (B=4, S=4096, DIN=384, DOUT=64).

out = softmax(cdist(q, k) / 8, axis=-1) @ v  with q/k/v = x @ W{q,k,v}.T

Sharding: 8 cores = 4 batches x 2 query-halves. Each core receives the
full x of its batch, host pre-transposed to x^T (bf16) with rows
reordered so its own query half comes first (softmax over keys is
permutation invariant). Every core runs the same SPMD program:
q = columns 0:2048, keys = all.

v2 restructuring vs the first working kernel (163.8us):
 - projections packed 2-wide: the stationary [Wq|Wk] / [Wk|Wv] pairs
   come for free as column slices of the same wT buffer, halving the
   projection matmul count for shared column ranges.
 - setup elementwise (copies, *-2, squares) moved to the ACT engine
   (copy/square live in every ACT table set, so they coexist with the
   Sqrt table at no switch cost); q2/k2 row sums via one 2-column
   ones-matmul over the [q;k] squared pair.
 - stage B (k-half projections) is interleaved chunk-by-chunk with
   phase-1 d2 tiles of the already-finished q-half region, keeping the
   PE warm (HAM) and removing the serial setup->phase1 boundary.
 - attention output accumulation (po) starts during phase 1 for tiles
   whose att came from the DVE composite path; a few DVE tiles are
   deferred into phase 2 so the DVE keeps working while ACT does Exp.

Per-core math (matmuls bf16 with fp32 accumulation):
  d2[j,i] = |q_i - k_j|^2 via one augmented matmul with contraction 66:
      lhsT = [-2*k^T; k2; 1; 0...]  (128 x 128 keys per tile)
      rhs  = [q^T; 1; q2; 0...]     (128 x 512)
  att = exp(sqrt(d2)/8) (unnormalized), two engine paths:
   - ScalarE: Sqrt(d2/64) -> fp16 buffer; after a scheduler barrier
     Exp with bias -2*ln(c0) -> bf16
   - VectorE: one custom DVE op (p(z)/c0)^2, p = minimax cubic of
     exp(sqrt(z)/16): the whole exp(sqrt(z)/8)/c0^2 in a single pass
  outT = [v; 1; 0...]^T @ att  (row 64 = softmax denominator, PSUM f32)
Final normalize outT[0:64]/outT[64] + transpose happen on the host.
"""

from contextlib import ExitStack

import ml_dtypes
import numpy as np

import concourse.bacc as bacc
import concourse.mybir as mybir
import concourse.tile as tile
from concourse import dve_ops
from concourse.dve_spec import Spec, Src0, C0, C1, C2, One, lower
from concourse.dve_uop import DveOpSpec
from concourse.bass_utils import run_bass_kernel_spmd

F32 = mybir.dt.float32
BF16 = mybir.dt.bfloat16
F16 = mybir.dt.float16
AF = mybir.ActivationFunctionType

B, S, DIN, DOUT = 4, 4096, 384, 64
M = S // 2        # query rows per core
KT = S // 128     # 32 key tiles
DC = DIN // 128   # 3 contraction chunks
NCORES = 8

# minimax cubic p for exp(sqrt(z)/16) on z in [32, 312], normalized by its
# constant term so the Horner tail can use the hardware One constant.
# att_dve = (p(z)/c0)^2 = exp(sqrt(z)/8)/c0^2; the ACT path matches the
# 1/c0^2 scale via a constant bias in its Exp (softmax is scale-invariant).
PA = 1.6518381642404523e-08
PB = -1.037933864407201e-05
PC = 0.006602996452846391
EXP_BIAS = -0.3424032850267295  # -2*ln(c0)

# tiles handled by the ACT sqrt/exp path; the rest use the DVE composite
N_ACT = 14
# DVE tiles whose d2/composite/po run in the phase-2 region so the DVE
# stays busy while ACT does its Exp passes
N_DEFER = 5


def _register_dve_op():
    name = "EXP_SQRT_SQ_ANT"
    if name in dve_ops._SUB_OPCODE_FOR_NAME:
        return next(op for op in dve_ops.OPS if op.name == name)
    t = ((Src0 * C0 + C1) * Src0 + C2) * Src0 + One
    body = t * t

    def ref(in0, in1, c0, c1, c2):
        tt = ((in0 * c0 + c1) * in0 + c2) * in0 + 1.0
        return tt * tt

    spec = Spec(body=body, reference=ref)
    row = max(dve_ops._SUB_OPCODE_FOR_NAME.values()) + 1
    assert row < 0x20
    dve_ops._SUB_OPCODE_FOR_NAME[name] = row
    shas = {}
    for ver in ("v3", "v4"):
        try:
            uops = lower(spec, ver=ver)
            shas[ver] = DveOpSpec(
                name=name, opcode=row, uops=uops, rd1_en=False
            ).sha(ver)
        except Exception:
            pass
    op = dve_ops.DveOp(name, spec, subdim=False, uops_sha=shas)
    dve_ops.OPS.append(op)
    dve_ops.CUSTOM_DVE_SPECS[name] = spec
    return op


EXP_OP = _register_dve_op()


def _is_act_tile(t):
    # spread ACT tiles evenly among the 32 key tiles
    return (t * N_ACT) % KT >= KT - N_ACT


def _body(tc, xt, wt, out):
    nc = tc.nc
    act_tiles = [t for t in range(KT) if _is_act_tile(t)]
    dve_tiles = [t for t in range(KT) if not _is_act_tile(t)]
    assert len(act_tiles) == N_ACT
    defer_tiles = dve_tiles[-N_DEFER:]
    early_dve = [t for t in dve_tiles if t not in defer_tiles]

    with ExitStack() as ctx:
        const_pool = ctx.enter_context(tc.tile_pool(name="const", bufs=1))
        # onesQ2: both columns select rows 0:64 (the q half of a squared
        # [q;k] pair) -> a [2,512] q2 result whose rows are identical, so
        # it can be copied to qT_aug[64:66] (base-partition-64 aligned);
        # row 64 is re-memset to 1 afterwards (the ones row).
        onesQ2 = const_pool.tile([128, 2], BF16)
        nc.vector.memset(onesQ2[:], 0.0)
        nc.vector.memset(onesQ2[0:64, :], 1.0)
        # onesK1: selects rows 64:128 (the k half) -> [1,512] k2 at
        # partition 0, copied to kT_aug[64:65].
        onesK1 = const_pool.tile([128, 1], BF16)
        nc.vector.memset(onesK1[:], 0.0)
        nc.vector.memset(onesK1[64:128, :], 1.0)
        ebias = const_pool.tile([128, 1], F32)
        nc.vector.memset(ebias[:], EXP_BIAS)

        main_pool = ctx.enter_context(tc.tile_pool(name="main", bufs=1))
        kT_aug = main_pool.tile([128, S], BF16)
        qT_aug = main_pool.tile([128, M], BF16)
        v_sb = main_pool.tile([128, KT, 128], BF16)
        # shared dist/att buffer: fp16 dist (ACT tiles) or bf16 att (DVE)
        buf = main_pool.tile([128, KT, M], F16)
        nc.vector.memset(kT_aug[64:128, :], 0.0)
        # rows 64:66 = 1.0; row 64 is overwritten by k2 per chunk, row 65
        # stays as the ones row (single-row memset at 65 is not a legal
        # base partition, hence the 2-row write)
        nc.vector.memset(kT_aug[64:66, :], 1.0)
        nc.vector.memset(qT_aug[64:128, :], 0.0)
        nc.gpsimd.memset(v_sb[:, :, 64:128], 0.0)
        nc.gpsimd.memset(v_sb[:, :, 64:65], 1.0)

        # PSUM pools: pp(3) + pb(1) + ps(4) = 8 banks during setup;
        # pp/pb close before po(4) opens, ps(4) stays -> 8 banks again.
        ps_pool = ctx.enter_context(tc.tile_pool(name="ps", bufs=2, space="PSUM"))

        def emit_d2_tile(t):
            for h in range(2):
                ps = ps_pool.tile([128, 1024], F32)
                base = h * 1024
                for s2 in range(2):
                    nc.tensor.matmul(
                        ps[:, s2 * 512 : (s2 + 1) * 512],
                        kT_aug[:, t * 128 : (t + 1) * 128],
                        qT_aug[:, base + s2 * 512 : base + (s2 + 1) * 512],
                        start=True,
                        stop=True,
                    )
                if _is_act_tile(t):
                    nc.scalar.activation(
                        buf[:, t, base : base + 1024], ps[:], AF.Sqrt,
                        scale=1.0 / 64.0,
                    )
                else:
                    nc.vector._custom_dve(
                        EXP_OP,
                        out=buf[:, t, base : base + 1024].bitcast(BF16),
                        in0=ps[:],
                        s0=PA,
                        s1=PB,
                        imm2=PC,
                    )

        po = None
        po_emitted = [0]
        PO_TOTAL = KT

        def emit_po_tile(t, att_ap):
            first = po_emitted[0] == 0
            last = po_emitted[0] == PO_TOTAL - 1
            for s2 in range(4):
                nc.tensor.matmul(
                    po[:, s2 * 512 : (s2 + 1) * 512],
                    v_sb[:, t, 0:128],
                    att_ap[:, s2 * 512 : (s2 + 1) * 512],
                    start=first,
                    stop=last,
                )
            po_emitted[0] += 1

        with ExitStack() as sctx:
            sb_pool = sctx.enter_context(tc.tile_pool(name="sbset", bufs=1))
            xk = sb_pool.tile([128, DC, M], BF16)
            wT = sb_pool.tile([128, DC, 3 * DOUT], BF16)
            # vT rows 0:64 = v of the q-half columns (from the v-only
            # matmul), rows 64:128 = v of the k-half columns (from the
            # [k;v] pair matmul) -- both partition-aligned copies.
            vT = sb_pool.tile([128, S], BF16)
            sq = sb_pool.tile([128, S], BF16, tag="sq")

            pp_pool = sctx.enter_context(
                tc.tile_pool(name="pp", bufs=2, space="PSUM")
            )
            pb_pool = sctx.enter_context(
                tc.tile_pool(name="pb", bufs=1, space="PSUM")
            )

            xt_r = xt.rearrange("(c p) s -> p c s", p=128)
            wt_r = wt.rearrange("(c p) w -> p c w", p=128)

            # ---------------- stage A: q-half projections ----------------
            with ExitStack() as actx:
                xq_pool = actx.enter_context(tc.tile_pool(name="xq", bufs=1))
                xq = xq_pool.tile([128, DC, M], BF16)

                # DMA: wT first (every matmul needs it), then x pieces,
                # alternating the two HWDGE rings; q-half pieces first.
                nc.scalar.dma_start(wT[:, :, :], wt_r[:, :, :])
                for p in range(2):
                    psl = slice(p * 1024, (p + 1) * 1024)
                    for c in range(DC):
                        eng = nc.sync if (p * DC + c) % 2 == 0 else nc.scalar
                        eng.dma_start(xq[:, c, psl], xt_r[:, c, psl])
                for p in range(2):
                    psl = slice(M + p * 1024, M + (p + 1) * 1024)
                    dsl = slice(p * 1024, (p + 1) * 1024)
                    for c in range(DC):
                        eng = nc.sync if (p * DC + c) % 2 == 1 else nc.scalar
                        eng.dma_start(xk[:, c, dsl], xt_r[:, c, psl])

                for ss in range(4):
                    sl = slice(ss * 512, (ss + 1) * 512)
                    ppA = pp_pool.tile([128, 512], F32, tag="p")
                    for c in range(DC):
                        nc.tensor.matmul(
                            ppA[:], wT[:, c, 0:128], xq[:, c, sl],
                            start=(c == 0), stop=(c == DC - 1),
                        )
                    # rows 0:64 = q, 64:128 = k
                    nc.scalar.copy(qT_aug[0:64, sl], ppA[0:64, :])
                    nc.scalar.mul(kT_aug[0:64, sl], ppA[64:128, :], -2.0)
                    nc.scalar.square(sq[:, sl], ppA[:])
                    pbq = pb_pool.tile([2, 512], F32, tag="b")
                    nc.tensor.matmul(
                        pbq[:], onesQ2[:, 0:2], sq[:, sl], start=True, stop=True
                    )
                    # both rows = q2; row 64 re-memset to 1 after the loop
                    nc.scalar.copy(qT_aug[64:66, sl], pbq[:])
                    pbk = pb_pool.tile([1, 512], F32, tag="b2")
                    nc.tensor.matmul(
                        pbk[:], onesK1[:, 0:1], sq[:, sl], start=True, stop=True
                    )
                    nc.scalar.copy(kT_aug[64:65, sl], pbk[0:1, :])

                    ppC = pp_pool.tile([64, 512], F32, tag="p")
                    for c in range(DC):
                        nc.tensor.matmul(
                            ppC[:], wT[:, c, 128:192], xq[:, c, sl],
                            start=(c == 0), stop=(c == DC - 1),
                        )
                    nc.scalar.copy(vT[0:64, sl], ppC[:])
                    for j in range(4):
                        t = ss * 4 + j
                        eng = nc.scalar if j % 2 == 0 else nc.sync
                        eng.dma_start_transpose(
                            v_sb[:, t, 0:64], vT[0:64, t * 128 : (t + 1) * 128]
                        )
                # restore the ones row (q2 copies wrote rows 64:66)
                nc.vector.memset(qT_aug[64:65, :], 1.0)

            # ------- stage B: k-half projections ||| phase-1 tiles 0..15 -------
            early_po_q = []  # DVE tiles whose att is ready for early po
            for ss in range(4):
                sl = slice(M + ss * 512, M + (ss + 1) * 512)
                dsl = slice(ss * 512, (ss + 1) * 512)
                ppA = pp_pool.tile([128, 512], F32, tag="p")
                for c in range(DC):
                    nc.tensor.matmul(
                        ppA[:], wT[:, c, 64:192], xk[:, c, dsl],
                        start=(c == 0), stop=(c == DC - 1),
                    )
                # rows 0:64 = k, 64:128 = v
                nc.scalar.mul(kT_aug[0:64, sl], ppA[0:64, :], -2.0)
                nc.scalar.copy(vT[64:128, sl], ppA[64:128, :])
                nc.scalar.square(sq[0:64, sl], ppA[0:64, :])
                pb = pb_pool.tile([1, 512], F32, tag="b")
                nc.tensor.matmul(
                    pb[:], onesQ2[0:64, 0:1], sq[0:64, sl], start=True, stop=True
                )
                nc.scalar.copy(kT_aug[64:65, sl], pb[0:1, :])
                for j in range(4):
                    t = 16 + ss * 4 + j
                    eng = nc.sync if j % 2 == 0 else nc.scalar
                    eng.dma_start_transpose(
                        v_sb[:, t, 0:64], vT[64:128, t * 128 : (t + 1) * 128]
                    )
                # phase-1 tiles over the q-half key region
                for j in range(4):
                    t = ss * 4 + j
                    emit_d2_tile(t)
                    if t in early_dve:
                        early_po_q.append(t)

        # setup pools closed (pp/pb PSUM freed) -> open po
        po_pool = ctx.enter_context(tc.tile_pool(name="po", bufs=1, space="PSUM"))
        po = po_pool.tile([128, M], F32)

        # ------- rest of phase 1: tiles 16..31 (minus deferred), with po
        # groups for ready DVE tiles interleaved to fill PE slack -------
        pending_po = list(early_po_q)
        for t in range(16, KT):
            if t in defer_tiles:
                continue
            emit_d2_tile(t)
            if not _is_act_tile(t):
                pending_po.append(t)
            if len(pending_po) > 2:
                c = pending_po.pop(0)
                emit_po_tile(c, buf[:, c, :].bitcast(BF16))
        for c in pending_po:
            emit_po_tile(c, buf[:, c, :].bitcast(BF16))

        tc.no_sync_barrier()  # all Sqrt before all Exp: one table switch

        # ---------------- phase 2: exp (ACT tiles) + deferred DVE ----------------
        with ExitStack() as p2ctx:
            att_pool = p2ctx.enter_context(tc.tile_pool(name="att", bufs=3))
            # interleave deferred DVE tiles among the ACT tiles
            seq = []
            di = 0
            for i, a in enumerate(act_tiles):
                seq.append(("act", a))
                if i % 3 == 2 and di < len(defer_tiles):
                    seq.append(("dve", defer_tiles[di]))
                    di += 1
            while di < len(defer_tiles):
                seq.append(("dve", defer_tiles[di]))
                di += 1
            for kind, t in seq:
                if kind == "act":
                    att = att_pool.tile([128, M], BF16)
                    nc.scalar.activation(
                        att[:], buf[:, t, :], AF.Exp, bias=ebias[:]
                    )
                    emit_po_tile(t, att[:])
                else:
                    emit_d2_tile(t)
                    emit_po_tile(t, buf[:, t, :].bitcast(BF16))
            assert po_emitted[0] == PO_TOTAL

            # -------- finish: copy outT[0:65] to SBUF, DMA out --------
            oT_pool = p2ctx.enter_context(tc.tile_pool(name="oT", bufs=1))
            oT = oT_pool.tile([65, M], F32)
            nc.vector.tensor_copy(oT[:], po[0:65, :])
            nc.sync.dma_start(out[:, :], oT[:])


_NC_CACHE = None


def build():
    global _NC_CACHE
    if _NC_CACHE is not None:
        return _NC_CACHE
    nc = bacc.Bacc("TRN2", target_bir_lowering=False, debug=False, num_devices=NCORES)
    xt_d = nc.declare_dram_parameter("xt", [DIN, S], BF16, isOutput=False)
    wt_d = nc.declare_dram_parameter("wt", [DIN, 3 * DOUT], BF16, isOutput=False)
    out_d = nc.declare_dram_parameter("out", [65, M], F32, isOutput=True)
    with tile.TileContext(nc) as tc:
        _body(tc, xt_d[:], wt_d[:], out_d[:])
    nc.compile()
    _NC_CACHE = nc
    return nc


def make_in_maps(x, Wq, Wk, Wv):
    bf16 = ml_dtypes.bfloat16
    wt = np.ascontiguousarray(
        np.concatenate(
            [np.asarray(W, np.float32).T for W in (Wq, Wk, Wv)], axis=1
        ).astype(bf16)
    )
    in_maps = []
    for c in range(NCORES):
        b, h = divmod(c, 2)
        xb = np.asarray(x[b], np.float32)
        xc = np.concatenate(
            [xb[h * M : (h + 1) * M], xb[(1 - h) * M : (2 - h) * M]], 0
        )
        in_maps.append({"xt": np.ascontiguousarray(xc.T.astype(bf16)), "wt": wt})
    return in_maps


def gather_out(results):
    out = np.zeros((B, S, DOUT), np.float32)
    for c in range(NCORES):
        b, h = divmod(c, 2)
        oT = np.asarray(results[c]["out"], np.float32)
        out[b, h * M : (h + 1) * M] = (oT[0:64] / oT[64:65]).T
    return out


def kernel(x, Wq, Wk, Wv):
    nc = build()
    in_maps = make_in_maps(x, Wq, Wk, Wv)
    res = run_bass_kernel_spmd(nc, in_maps, core_ids=list(range(NCORES)))
    return gather_out(res.results)


# revision 21
# speedup vs baseline: 1.3622x; 1.2887x over previous
"""Trainium2 Bass kernel for AttentionL2 (B=4, S=4096, DIN=384, DOUT=64).

out = softmax(cdist(q, k) / 8, axis=-1) @ v  with q/k/v = x @ W{q,k,v}.T

Sharding: 8 cores = 4 batches x 2 query-halves. Each core receives the
full x of its batch, host pre-transposed to x^T (bf16) with rows
reordered so its own query half comes first (softmax over keys is
permutation invariant). Every core runs the same SPMD program:
q = columns 0:2048, keys = all.

v2 restructuring vs the first working kernel (163.8us):
 - projections packed 2-wide: the stationary [Wq|Wk] / [Wk|Wv] pairs
   come for free as column slices of the same wT buffer, halving the
   projection matmul count for shared column ranges.
 - setup elementwise (copies, *-2, squares) moved to the ACT engine
   (copy/square live in every ACT table set, so they coexist with the
   Sqrt table at no switch cost); q2/k2 row sums via one 2-column
   ones-matmul over the [q;k] squared pair.
 - stage B (k-half projections) is interleaved chunk-by-chunk with
   phase-1 d2 tiles of the already-finished q-half region, keeping the
   PE warm (HAM) and removing the serial setup->phase1 boundary.
 - attention output accumulation (po) starts during phase 1 for tiles
   whose att came from the DVE composite path; a few DVE tiles are
   deferred into phase 2 so the DVE keeps working while ACT does Exp.

Per-core math (matmuls bf16 with fp32 accumulation):
  d2[j,i] = |q_i - k_j|^2 via one augmented matmul with contraction 66:
      lhsT = [-2*k^T; k2; 1; 0...]  (128 x 128 keys per tile)
      rhs  = [q^T; 1; q2; 0...]     (128 x 512)
  att = exp(sqrt(d2)/8) (unnormalized), two engine paths:
   - ScalarE: Sqrt(d2/64) -> fp16 buffer; after a scheduler barrier
     Exp with bias -2*ln(c0) -> bf16
   - VectorE: one custom DVE op (p(z)/c0)^2, p = minimax cubic of
     exp(sqrt(z)/16): the whole exp(sqrt(z)/8)/c0^2 in a single pass
  outT = [v; 1; 0...]^T @ att  (row 64 = softmax denominator, PSUM f32)
Final normalize outT[0:64]/outT[64] + transpose happen on the host.
"""

from contextlib import ExitStack

import ml_dtypes
import numpy as np

import concourse.bacc as bacc
import concourse.mybir as mybir
import concourse.tile as tile
from concourse import dve_ops
from concourse.dve_spec import Spec, Src0, C0, C1, C2, One, lower
from concourse.dve_uop import DveOpSpec
from concourse.bass_utils import run_bass_kernel_spmd

F32 = mybir.dt.float32
BF16 = mybir.dt.bfloat16
F16 = mybir.dt.float16
AF = mybir.ActivationFunctionType

B, S, DIN, DOUT = 4, 4096, 384, 64
M = S // 2        # query rows per core
KT = S // 128     # 32 key tiles
DC = DIN // 128   # 3 contraction chunks
NCORES = 8

# minimax cubic p for exp(sqrt(z)/16) on z in [32, 312], normalized by its
# constant term so the Horner tail can use the hardware One constant.
# att_dve = (p(z)/c0)^2 = exp(sqrt(z)/8)/c0^2; the ACT path matches the
# 1/c0^2 scale via a constant bias in its Exp (softmax is scale-invariant).
PA = 1.6518381642404523e-08
PB = -1.037933864407201e-05
PC = 0.006602996452846391
EXP_BIAS = -0.3424032850267295  # -2*ln(c0)

# tiles handled by the ACT sqrt/exp path (adjacent pairs so Exp can be
# batched as one [128,2,M] call); the rest use the DVE composite
ACT_TILES = (2, 3, 8, 9, 14, 15, 20, 21, 26, 27)
# DVE tiles whose d2/composite/po run in the phase-2 region so the DVE
# stays busy while ACT does its Exp passes
N_DEFER = 7


def _register_dve_op():
    name = "EXP_SQRT_SQ_ANT"
    if name in dve_ops._SUB_OPCODE_FOR_NAME:
        return next(op for op in dve_ops.OPS if op.name == name)
    t = ((Src0 * C0 + C1) * Src0 + C2) * Src0 + One
    body = t * t

    def ref(in0, in1, c0, c1, c2):
        tt = ((in0 * c0 + c1) * in0 + c2) * in0 + 1.0
        return tt * tt

    spec = Spec(body=body, reference=ref)
    row = max(dve_ops._SUB_OPCODE_FOR_NAME.values()) + 1
    assert row < 0x20
    dve_ops._SUB_OPCODE_FOR_NAME[name] = row
    shas = {}
    for ver in ("v3", "v4"):
        try:
            uops = lower(spec, ver=ver)
            shas[ver] = DveOpSpec(
                name=name, opcode=row, uops=uops, rd1_en=False
            ).sha(ver)
        except Exception:
            pass
    op = dve_ops.DveOp(name, spec, subdim=False, uops_sha=shas)
    dve_ops.OPS.append(op)
    dve_ops.CUSTOM_DVE_SPECS[name] = spec
    return op


EXP_OP = _register_dve_op()


def _is_act_tile(t):
    return t in ACT_TILES


def _body(tc, xt, wt, out):
    nc = tc.nc
    act_tiles = [t for t in range(KT) if _is_act_tile(t)]
    dve_tiles = [t for t in range(KT) if not _is_act_tile(t)]
    defer_tiles = dve_tiles[-N_DEFER:]
    early_dve = [t for t in dve_tiles if t not in defer_tiles]

    with ExitStack() as ctx:
        const_pool = ctx.enter_context(tc.tile_pool(name="const", bufs=1))
        # onesQ2: both columns select rows 0:64 (the q half of a squared
        # [q;k] pair) -> a [2,512] q2 result whose rows are identical, so
        # it can be copied to qT_aug[64:66] (base-partition-64 aligned);
        # row 64 is re-memset to 1 afterwards (the ones row).
        onesQ2 = const_pool.tile([128, 2], BF16)
        nc.vector.memset(onesQ2[:], 0.0)
        nc.vector.memset(onesQ2[0:64, :], 1.0)
        # onesK1: selects rows 64:128 (the k half) -> [1,512] k2 at
        # partition 0, copied to kT_aug[64:65].
        onesK1 = const_pool.tile([128, 1], BF16)
        nc.vector.memset(onesK1[:], 0.0)
        nc.vector.memset(onesK1[64:128, :], 1.0)
        ebias = const_pool.tile([128, 1], F32)
        nc.vector.memset(ebias[:], EXP_BIAS)

        main_pool = ctx.enter_context(tc.tile_pool(name="main", bufs=1))
        kT_aug = main_pool.tile([128, S], BF16)
        qT_aug = main_pool.tile([128, M], BF16)
        v_sb = main_pool.tile([128, KT, 128], BF16)
        # shared dist/att buffer: fp16 dist (ACT tiles) or bf16 att (DVE)
        buf = main_pool.tile([128, KT, M], F16)
        nc.vector.memset(kT_aug[64:128, :], 0.0)
        # rows 64:66 = 1.0; row 64 is overwritten by k2 per chunk, row 65
        # stays as the ones row (single-row memset at 65 is not a legal
        # base partition, hence the 2-row write)
        nc.vector.memset(kT_aug[64:66, :], 1.0)
        nc.vector.memset(qT_aug[64:128, :], 0.0)
        nc.gpsimd.memset(v_sb[:, :, 64:128], 0.0)
        nc.gpsimd.memset(v_sb[:, :, 64:65], 1.0)

        # PSUM pools: pp(3) + pb(1) + ps(4) = 8 banks during setup;
        # pp/pb close before po(4) opens, ps(4) stays -> 8 banks again.
        ps_pool = ctx.enter_context(tc.tile_pool(name="ps", bufs=2, space="PSUM"))

        def emit_d2_tile(t):
            for h in range(2):
                ps = ps_pool.tile([128, 1024], F32)
                base = h * 1024
                for s2 in range(2):
                    nc.tensor.matmul(
                        ps[:, s2 * 512 : (s2 + 1) * 512],
                        kT_aug[:, t * 128 : (t + 1) * 128],
                        qT_aug[:, base + s2 * 512 : base + (s2 + 1) * 512],
                        start=True,
                        stop=True,
                    )
                if _is_act_tile(t):
                    nc.scalar.activation(
                        buf[:, t, base : base + 1024], ps[:], AF.Sqrt,
                        scale=1.0 / 64.0,
                    )
                else:
                    nc.vector._custom_dve(
                        EXP_OP,
                        out=buf[:, t, base : base + 1024].bitcast(BF16),
                        in0=ps[:],
                        s0=PA,
                        s1=PB,
                        imm2=PC,
                    )

        po = None
        po_emitted = [0]
        PO_TOTAL = KT

        def emit_po_tile(t, att_ap):
            first = po_emitted[0] == 0
            last = po_emitted[0] == PO_TOTAL - 1
            for s2 in range(4):
                nc.tensor.matmul(
                    po[:, s2 * 512 : (s2 + 1) * 512],
                    v_sb[:, t, 0:128],
                    att_ap[:, s2 * 512 : (s2 + 1) * 512],
                    start=first,
                    stop=last,
                )
            po_emitted[0] += 1

        with ExitStack() as sctx:
            sb_pool = sctx.enter_context(tc.tile_pool(name="sbset", bufs=1))
            xk = sb_pool.tile([128, DC, M], BF16)
            wT = sb_pool.tile([128, DC, 3 * DOUT], BF16)
            # vT rows 0:64 = v of the q-half columns (from the v-only
            # matmul), rows 64:128 = v of the k-half columns (from the
            # [k;v] pair matmul) -- both partition-aligned copies.
            vT = sb_pool.tile([128, S], BF16)
            sq = sb_pool.tile([128, S], BF16, tag="sq")

            pp_pool = sctx.enter_context(
                tc.tile_pool(name="pp", bufs=2, space="PSUM")
            )
            pb_pool = sctx.enter_context(
                tc.tile_pool(name="pb", bufs=2, space="PSUM")
            )

            xt_r = xt.rearrange("(c p) s -> p c s", p=128)
            wt_r = wt.rearrange("(c p) w -> p c w", p=128)

            # ---------------- stage A: q-half projections ----------------
            with ExitStack() as actx:
                xq_pool = actx.enter_context(tc.tile_pool(name="xq", bufs=1))
                xq = xq_pool.tile([128, DC, M], BF16)

                # DMA policy: the ACT queue carries NO dma (a HWDGE issue
                # occupies the queue for the transfer duration and starves
                # the setup copies). x goes via GPSIMD/SWDGE; wT, the 32
                # v transposes and the output ride the sync HWDGE ring.
                nc.sync.dma_start(wT[:, :, :], wt_r[:, :, :])
                for p in range(2):
                    psl = slice(p * 1024, (p + 1) * 1024)
                    for c in range(DC):
                        nc.gpsimd.dma_start(xq[:, c, psl], xt_r[:, c, psl])
                for p in range(2):
                    psl = slice(M + p * 1024, M + (p + 1) * 1024)
                    dsl = slice(p * 1024, (p + 1) * 1024)
                    for c in range(DC):
                        nc.gpsimd.dma_start(xk[:, c, dsl], xt_r[:, c, psl])

                for ss in range(4):
                    sl = slice(ss * 512, (ss + 1) * 512)
                    ppA = pp_pool.tile([128, 512], F32, tag="p")
                    for c in range(DC):
                        nc.tensor.matmul(
                            ppA[:], wT[:, c, 0:128], xq[:, c, sl],
                            start=(c == 0), stop=(c == DC - 1),
                        )
                    # rows 0:64 = q, 64:128 = k
                    nc.scalar.copy(qT_aug[0:64, sl], ppA[0:64, :])
                    nc.scalar.mul(kT_aug[0:64, sl], ppA[64:128, :], -2.0)
                    nc.scalar.square(sq[:, sl], ppA[:])

                    ppC = pp_pool.tile([64, 512], F32, tag="p")
                    for c in range(DC):
                        nc.tensor.matmul(
                            ppC[:], wT[:, c, 128:192], xq[:, c, sl],
                            start=(c == 0), stop=(c == DC - 1),
                        )
                    nc.scalar.copy(vT[0:64, sl], ppC[:])
                    for j in range(4):
                        t = ss * 4 + j
                        nc.sync.dma_start_transpose(
                            v_sb[:, t, 0:64], vT[0:64, t * 128 : (t + 1) * 128]
                        )

                # q2/k2 sums deferred out of the per-chunk chain: by now
                # the squares are done, so the PE never stalls on ACT here
                for ss in range(4):
                    sl = slice(ss * 512, (ss + 1) * 512)
                    pbq = pb_pool.tile([2, 512], F32, tag="b")
                    nc.tensor.matmul(
                        pbq[:], onesQ2[:, 0:2], sq[:, sl], start=True, stop=True
                    )
                    # both rows = q2; row 64 re-memset to 1 after the loop
                    nc.scalar.copy(qT_aug[64:66, sl], pbq[:])
                    pbk = pb_pool.tile([1, 512], F32, tag="b")
                    nc.tensor.matmul(
                        pbk[:], onesK1[:, 0:1], sq[:, sl], start=True, stop=True
                    )
                    nc.scalar.copy(kT_aug[64:65, sl], pbk[0:1, :])
                # restore the ones row (q2 copies wrote rows 64:66)
                nc.vector.memset(qT_aug[64:65, :], 1.0)

            # ------- stage B: k-half projections ||| phase-1 tiles 0..15 -------
            early_po_q = []  # DVE tiles whose att is ready for early po

            def emit_b_sum(ss):
                sl = slice(M + ss * 512, M + (ss + 1) * 512)
                pb = pb_pool.tile([1, 512], F32, tag="b")
                nc.tensor.matmul(
                    pb[:], onesQ2[0:64, 0:1], sq[0:64, sl], start=True, stop=True
                )
                nc.scalar.copy(kT_aug[64:65, sl], pb[0:1, :])

            for ss in range(4):
                sl = slice(M + ss * 512, M + (ss + 1) * 512)
                dsl = slice(ss * 512, (ss + 1) * 512)
                ppA = pp_pool.tile([128, 512], F32, tag="p")
                for c in range(DC):
                    nc.tensor.matmul(
                        ppA[:], wT[:, c, 64:192], xk[:, c, dsl],
                        start=(c == 0), stop=(c == DC - 1),
                    )
                # rows 0:64 = k, 64:128 = v
                nc.scalar.mul(kT_aug[0:64, sl], ppA[0:64, :], -2.0)
                nc.scalar.copy(vT[64:128, sl], ppA[64:128, :])
                nc.scalar.square(sq[0:64, sl], ppA[0:64, :])
                # k2 sum for the PREVIOUS chunk (its square is long done,
                # so the PE does not stall on the ACT chain)
                if ss > 0:
                    emit_b_sum(ss - 1)
                for j in range(4):
                    t = 16 + ss * 4 + j
                    nc.sync.dma_start_transpose(
                        v_sb[:, t, 0:64], vT[64:128, t * 128 : (t + 1) * 128]
                    )
                # phase-1 tiles over the q-half key region
                for j in range(4):
                    t = ss * 4 + j
                    emit_d2_tile(t)
                    if t in early_dve:
                        early_po_q.append(t)
            emit_b_sum(3)

        # setup pools closed (pp/pb PSUM freed) -> open po
        po_pool = ctx.enter_context(tc.tile_pool(name="po", bufs=1, space="PSUM"))
        po = po_pool.tile([128, M], F32)

        # ------- rest of phase 1: tiles 16..31 (minus deferred), with po
        # groups for ready DVE tiles interleaved to fill PE slack -------
        pending_po = list(early_po_q)
        for t in range(16, KT):
            if t in defer_tiles:
                continue
            emit_d2_tile(t)
            if not _is_act_tile(t):
                pending_po.append(t)
            if len(pending_po) > 2:
                c = pending_po.pop(0)
                emit_po_tile(c, buf[:, c, :].bitcast(BF16))
        for c in pending_po:
            emit_po_tile(c, buf[:, c, :].bitcast(BF16))

        tc.no_sync_barrier()  # all Sqrt before all Exp: one table switch

        # ---------------- phase 2: exp (ACT tiles) + deferred DVE ----------------
        with ExitStack() as p2ctx:
            att_pool = p2ctx.enter_context(tc.tile_pool(name="att", bufs=3))
            # ACT tiles come in adjacent pairs -> one batched Exp per pair;
            # deferred DVE tiles interleave so the DVE works through phase 2
            pairs = [
                (act_tiles[i], act_tiles[i + 1])
                for i in range(0, len(act_tiles), 2)
            ]
            dq = list(defer_tiles)
            defer_counts = [2, 1, 1, 1, 2] if len(dq) == 7 else None
            for pi, (t0, t1) in enumerate(pairs):
                att2 = att_pool.tile([128, 2, M], BF16)
                nc.scalar.activation(
                    att2[:], buf[:, t0 : t0 + 2, :], AF.Exp, bias=ebias[:]
                )
                emit_po_tile(t0, att2[:, 0, :])
                emit_po_tile(t1, att2[:, 1, :])
                n_d = defer_counts[pi] if defer_counts else (1 if dq else 0)
                for _ in range(n_d):
                    if dq:
                        t = dq.pop(0)
                        emit_d2_tile(t)
                        emit_po_tile(t, buf[:, t, :].bitcast(BF16))
            while dq:
                t = dq.pop(0)
                emit_d2_tile(t)
                emit_po_tile(t, buf[:, t, :].bitcast(BF16))
            assert po_emitted[0] == PO_TOTAL

            # -------- finish: copy outT[0:65] to SBUF, DMA out --------
            # split across DVE and ACT to halve the tail
            oT_pool = p2ctx.enter_context(tc.tile_pool(name="oT", bufs=1))
            oT = oT_pool.tile([65, M], F32)
            nc.vector.tensor_copy(oT[:, 0 : M // 2], po[0:65, 0 : M // 2])
            nc.scalar.copy(oT[:, M // 2 : M], po[0:65, M // 2 : M])
            nc.sync.dma_start(out[:, 0 : M // 2], oT[:, 0 : M // 2])
            nc.sync.dma_start(out[:, M // 2 : M], oT[:, M // 2 : M])


_NC_CACHE = None


def build():
    global _NC_CACHE
    if _NC_CACHE is not None:
        return _NC_CACHE
    nc = bacc.Bacc("TRN2", target_bir_lowering=False, debug=False, num_devices=NCORES)
    xt_d = nc.declare_dram_parameter("xt", [DIN, S], BF16, isOutput=False)
    wt_d = nc.declare_dram_parameter("wt", [DIN, 3 * DOUT], BF16, isOutput=False)
    out_d = nc.declare_dram_parameter("out", [65, M], F32, isOutput=True)
    with tile.TileContext(nc) as tc:
        _body(tc, xt_d[:], wt_d[:], out_d[:])
    nc.compile()
    _NC_CACHE = nc
    return nc


def make_in_maps(x, Wq, Wk, Wv):
    bf16 = ml_dtypes.bfloat16
    wt = np.ascontiguousarray(
        np.concatenate(
            [np.asarray(W, np.float32).T for W in (Wq, Wk, Wv)], axis=1
        ).astype(bf16)
    )
    in_maps = []
    for c in range(NCORES):
        b, h = divmod(c, 2)
        xb = np.asarray(x[b], np.float32)
        xc = np.concatenate(
            [xb[h * M : (h + 1) * M], xb[(1 - h) * M : (2 - h) * M]], 0
        )
        in_maps.append({"xt": np.ascontiguousarray(xc.T.astype(bf16)), "wt": wt})
    return in_maps


def gather_out(results):
    out = np.zeros((B, S, DOUT), np.float32)
    for c in range(NCORES):
        b, h = divmod(c, 2)
        oT = np.asarray(results[c]["out"], np.float32)
        out[b, h * M : (h + 1) * M] = (oT[0:64] / oT[64:65]).T
    return out


def kernel(x, Wq, Wk, Wv):
    nc = build()
    in_maps = make_in_maps(x, Wq, Wk, Wv)
    res = run_bass_kernel_spmd(nc, in_maps, core_ids=list(range(NCORES)))
    return gather_out(res.results)


# revision 23
# speedup vs baseline: 1.4278x; 1.0481x over previous
"""Trainium2 Bass kernel for AttentionL2 (B=4, S=4096, DIN=384, DOUT=64).

out = softmax(cdist(q, k) / 8, axis=-1) @ v  with q/k/v = x @ W{q,k,v}.T

Sharding: 8 cores = 4 batches x 2 query-halves. Each core receives the
full x of its batch, host pre-transposed to x^T (bf16) with rows
reordered so its own query half comes first (softmax over keys is
permutation invariant). Every core runs the same SPMD program:
q = columns 0:2048, keys = all.

v2 restructuring vs the first working kernel (163.8us):
 - projections packed 2-wide: the stationary [Wq|Wk] / [Wk|Wv] pairs
   come for free as column slices of the same wT buffer, halving the
   projection matmul count for shared column ranges.
 - setup elementwise (copies, *-2, squares) moved to the ACT engine
   (copy/square live in every ACT table set, so they coexist with the
   Sqrt table at no switch cost); q2/k2 row sums via one 2-column
   ones-matmul over the [q;k] squared pair.
 - stage B (k-half projections) is interleaved chunk-by-chunk with
   phase-1 d2 tiles of the already-finished q-half region, keeping the
   PE warm (HAM) and removing the serial setup->phase1 boundary.
 - attention output accumulation (po) starts during phase 1 for tiles
   whose att came from the DVE composite path; a few DVE tiles are
   deferred into phase 2 so the DVE keeps working while ACT does Exp.

Per-core math (matmuls bf16 with fp32 accumulation):
  d2[j,i] = |q_i - k_j|^2 via one augmented matmul with contraction 66:
      lhsT = [-2*k^T; k2; 1; 0...]  (128 x 128 keys per tile)
      rhs  = [q^T; 1; q2; 0...]     (128 x 512)
  att = exp(sqrt(d2)/8) (unnormalized), two engine paths:
   - ScalarE: Sqrt(d2/64) -> fp16 buffer; after a scheduler barrier
     Exp with bias -2*ln(c0) -> bf16
   - VectorE: one custom DVE op (p(z)/c0)^2, p = minimax cubic of
     exp(sqrt(z)/16): the whole exp(sqrt(z)/8)/c0^2 in a single pass
  outT = [v; 1; 0...]^T @ att  (row 64 = softmax denominator, PSUM f32)
Final normalize outT[0:64]/outT[64] + transpose happen on the host.
"""

from contextlib import ExitStack

import ml_dtypes
import numpy as np

import concourse.bacc as bacc
import concourse.mybir as mybir
import concourse.tile as tile
from concourse import dve_ops
from concourse.dve_spec import Spec, Src0, C0, C1, C2, One, lower
from concourse.dve_uop import DveOpSpec
from concourse.bass_utils import run_bass_kernel_spmd

F32 = mybir.dt.float32
BF16 = mybir.dt.bfloat16
F16 = mybir.dt.float16
AF = mybir.ActivationFunctionType

B, S, DIN, DOUT = 4, 4096, 384, 64
M = S // 2        # query rows per core
KT = S // 128     # 32 key tiles
DC = DIN // 128   # 3 contraction chunks
NCORES = 8

# minimax cubic p for exp(sqrt(z)/16) on z in [32, 312], normalized by its
# constant term so the Horner tail can use the hardware One constant.
# att_dve = (p(z)/c0)^2 = exp(sqrt(z)/8)/c0^2; the ACT path matches the
# 1/c0^2 scale via a constant bias in its Exp (softmax is scale-invariant).
PA = 1.6518381642404523e-08
PB = -1.037933864407201e-05
PC = 0.006602996452846391
EXP_BIAS = -0.3424032850267295  # -2*ln(c0)

# tiles handled by the ACT sqrt/exp path (adjacent pairs so Exp can be
# batched as one [128,2,M] call); the rest use the DVE composite
ACT_TILES = (2, 3, 8, 9, 14, 15, 20, 21, 26, 27)
# DVE tiles whose d2/composite/po run in the phase-2 region so the DVE
# stays busy while ACT does its Exp passes
N_DEFER = 7


def _register_dve_op():
    name = "EXP_SQRT_SQ_ANT"
    if name in dve_ops._SUB_OPCODE_FOR_NAME:
        return next(op for op in dve_ops.OPS if op.name == name)
    t = ((Src0 * C0 + C1) * Src0 + C2) * Src0 + One
    body = t * t

    def ref(in0, in1, c0, c1, c2):
        tt = ((in0 * c0 + c1) * in0 + c2) * in0 + 1.0
        return tt * tt

    spec = Spec(body=body, reference=ref)
    row = max(dve_ops._SUB_OPCODE_FOR_NAME.values()) + 1
    assert row < 0x20
    dve_ops._SUB_OPCODE_FOR_NAME[name] = row
    shas = {}
    for ver in ("v3", "v4"):
        try:
            uops = lower(spec, ver=ver)
            shas[ver] = DveOpSpec(
                name=name, opcode=row, uops=uops, rd1_en=False
            ).sha(ver)
        except Exception:
            pass
    op = dve_ops.DveOp(name, spec, subdim=False, uops_sha=shas)
    dve_ops.OPS.append(op)
    dve_ops.CUSTOM_DVE_SPECS[name] = spec
    return op


EXP_OP = _register_dve_op()


def _is_act_tile(t):
    return t in ACT_TILES


def _body(tc, xt, wt, out):
    nc = tc.nc
    act_tiles = [t for t in range(KT) if _is_act_tile(t)]
    dve_tiles = [t for t in range(KT) if not _is_act_tile(t)]
    defer_tiles = dve_tiles[-N_DEFER:]
    early_dve = [t for t in dve_tiles if t not in defer_tiles]

    with ExitStack() as ctx:
        const_pool = ctx.enter_context(tc.tile_pool(name="const", bufs=1))
        # onesQ2: both columns select rows 0:64 (the q half of a squared
        # [q;k] pair) -> a [2,512] q2 result whose rows are identical, so
        # it can be copied to qT_aug[64:66] (base-partition-64 aligned);
        # row 64 is re-memset to 1 afterwards (the ones row).
        onesQ2 = const_pool.tile([128, 2], BF16)
        nc.vector.memset(onesQ2[:], 0.0)
        nc.vector.memset(onesQ2[0:64, :], 1.0)
        # onesK1: selects rows 64:128 (the k half) -> [1,512] k2 at
        # partition 0, copied to kT_aug[64:65].
        onesK1 = const_pool.tile([128, 1], BF16)
        nc.vector.memset(onesK1[:], 0.0)
        nc.vector.memset(onesK1[64:128, :], 1.0)
        ebias = const_pool.tile([128, 1], F32)
        nc.vector.memset(ebias[:], EXP_BIAS)

        main_pool = ctx.enter_context(tc.tile_pool(name="main", bufs=1))
        kT_aug = main_pool.tile([128, S], BF16)
        qT_aug = main_pool.tile([128, M], BF16)
        v_sb = main_pool.tile([128, KT, 128], BF16)
        # shared dist/att buffer: fp16 dist (ACT tiles) or bf16 att (DVE)
        buf = main_pool.tile([128, KT, M], F16)
        nc.vector.memset(kT_aug[64:128, :], 0.0)
        # rows 64:66 = 1.0; row 64 is overwritten by k2 per chunk, row 65
        # stays as the ones row (single-row memset at 65 is not a legal
        # base partition, hence the 2-row write)
        nc.vector.memset(kT_aug[64:66, :], 1.0)
        nc.vector.memset(qT_aug[64:128, :], 0.0)
        nc.gpsimd.memset(v_sb[:, :, 64:128], 0.0)
        nc.gpsimd.memset(v_sb[:, :, 64:65], 1.0)

        # PSUM pools: pp(3) + pb(1) + ps(4) = 8 banks during setup;
        # pp/pb close before po(4) opens, ps(4) stays -> 8 banks again.
        ps_pool = ctx.enter_context(tc.tile_pool(name="ps", bufs=2, space="PSUM"))

        def emit_d2_tile(t):
            for h in range(2):
                ps = ps_pool.tile([128, 1024], F32)
                base = h * 1024
                for s2 in range(2):
                    nc.tensor.matmul(
                        ps[:, s2 * 512 : (s2 + 1) * 512],
                        kT_aug[:, t * 128 : (t + 1) * 128],
                        qT_aug[:, base + s2 * 512 : base + (s2 + 1) * 512],
                        start=True,
                        stop=True,
                    )
                if _is_act_tile(t):
                    nc.scalar.activation(
                        buf[:, t, base : base + 1024], ps[:], AF.Sqrt,
                        scale=1.0 / 64.0,
                    )
                else:
                    nc.vector._custom_dve(
                        EXP_OP,
                        out=buf[:, t, base : base + 1024].bitcast(BF16),
                        in0=ps[:],
                        s0=PA,
                        s1=PB,
                        imm2=PC,
                    )

        po = None
        po_emitted = [0]
        PO_TOTAL = KT

        def emit_po_tile(t, att_ap):
            first = po_emitted[0] == 0
            last = po_emitted[0] == PO_TOTAL - 1
            for s2 in range(4):
                nc.tensor.matmul(
                    po[:, s2 * 512 : (s2 + 1) * 512],
                    v_sb[:, t, 0:128],
                    att_ap[:, s2 * 512 : (s2 + 1) * 512],
                    start=first,
                    stop=last,
                )
            po_emitted[0] += 1

        with ExitStack() as sctx:
            sb_pool = sctx.enter_context(tc.tile_pool(name="sbset", bufs=1))
            xk = sb_pool.tile([128, DC, M], BF16)
            wT = sb_pool.tile([128, DC, 3 * DOUT], BF16)
            # vT rows 0:64 = v of the q-half columns (from the v-only
            # matmul), rows 64:128 = v of the k-half columns (from the
            # [k;v] pair matmul) -- both partition-aligned copies.
            vT = sb_pool.tile([128, S], BF16)
            sq = sb_pool.tile([128, S], BF16, tag="sq")

            pp_pool = sctx.enter_context(
                tc.tile_pool(name="pp", bufs=2, space="PSUM")
            )
            pb_pool = sctx.enter_context(
                tc.tile_pool(name="pb", bufs=2, space="PSUM")
            )

            xt_r = xt.rearrange("(c p) s -> p c s", p=128)
            wt_r = wt.rearrange("(c p) w -> p c w", p=128)

            # ---------------- stage A: q-half projections ----------------
            with ExitStack() as actx:
                xq_pool = actx.enter_context(tc.tile_pool(name="xq", bufs=1))
                xq = xq_pool.tile([128, DC, M], BF16)

                # DMA policy: the ACT queue carries almost no dma (a HWDGE
                # issue occupies the queue for the transfer duration and
                # starves the setup copies) -- only the small wT load, done
                # by ~2.5us. The first-needed x pieces ride the sync ring;
                # the later halves go via GPSIMD/SWDGE in parallel. The 32
                # v transposes and the output also use sync.
                nc.scalar.dma_start(wT[:, :, :], wt_r[:, :, :])
                for c in range(DC):
                    nc.sync.dma_start(xq[:, c, 0:1024], xt_r[:, c, 0:1024])
                for c in range(DC):
                    nc.gpsimd.dma_start(
                        xq[:, c, 1024:2048], xt_r[:, c, 1024:2048]
                    )
                for c in range(DC):
                    nc.sync.dma_start(xk[:, c, 0:1024], xt_r[:, c, M : M + 1024])
                for c in range(DC):
                    nc.gpsimd.dma_start(
                        xk[:, c, 1024:2048], xt_r[:, c, M + 1024 : S]
                    )

                def emit_a_sum(ss):
                    # q2/k2 sums one chunk behind the projection chain: the
                    # squares are already done, so the PE never stalls here,
                    # and each 512-column slice of the aug rows unblocks its
                    # d2 tiles independently.
                    sl = slice(ss * 512, (ss + 1) * 512)
                    pbq = pb_pool.tile([2, 512], F32, tag="b")
                    nc.tensor.matmul(
                        pbq[:], onesQ2[:, 0:2], sq[:, sl], start=True, stop=True
                    )
                    # both rows = q2; row 64 re-memset to 1 per chunk
                    nc.scalar.copy(qT_aug[64:66, sl], pbq[:])
                    nc.vector.memset(qT_aug[64:65, sl], 1.0)
                    pbk = pb_pool.tile([1, 512], F32, tag="b")
                    nc.tensor.matmul(
                        pbk[:], onesK1[:, 0:1], sq[:, sl], start=True, stop=True
                    )
                    nc.scalar.copy(kT_aug[64:65, sl], pbk[0:1, :])

                for ss in range(4):
                    sl = slice(ss * 512, (ss + 1) * 512)
                    ppA = pp_pool.tile([128, 512], F32, tag="p")
                    for c in range(DC):
                        nc.tensor.matmul(
                            ppA[:], wT[:, c, 0:128], xq[:, c, sl],
                            start=(c == 0), stop=(c == DC - 1),
                        )
                    # rows 0:64 = q, 64:128 = k
                    nc.scalar.copy(qT_aug[0:64, sl], ppA[0:64, :])
                    nc.scalar.mul(kT_aug[0:64, sl], ppA[64:128, :], -2.0)
                    nc.scalar.square(sq[:, sl], ppA[:])

                    ppC = pp_pool.tile([64, 512], F32, tag="p")
                    for c in range(DC):
                        nc.tensor.matmul(
                            ppC[:], wT[:, c, 128:192], xq[:, c, sl],
                            start=(c == 0), stop=(c == DC - 1),
                        )
                    nc.scalar.copy(vT[0:64, sl], ppC[:])
                    for j in range(4):
                        t = ss * 4 + j
                        nc.sync.dma_start_transpose(
                            v_sb[:, t, 0:64], vT[0:64, t * 128 : (t + 1) * 128]
                        )
                    if ss > 0:
                        emit_a_sum(ss - 1)
                emit_a_sum(3)

            # ------- stage B: k-half projections ||| phase-1 tiles 0..15 -------
            early_po_q = []  # DVE tiles whose att is ready for early po

            def emit_b_sum(ss):
                sl = slice(M + ss * 512, M + (ss + 1) * 512)
                pb = pb_pool.tile([1, 512], F32, tag="b")
                nc.tensor.matmul(
                    pb[:], onesQ2[0:64, 0:1], sq[0:64, sl], start=True, stop=True
                )
                nc.scalar.copy(kT_aug[64:65, sl], pb[0:1, :])

            for ss in range(4):
                sl = slice(M + ss * 512, M + (ss + 1) * 512)
                dsl = slice(ss * 512, (ss + 1) * 512)
                ppA = pp_pool.tile([128, 512], F32, tag="p")
                for c in range(DC):
                    nc.tensor.matmul(
                        ppA[:], wT[:, c, 64:192], xk[:, c, dsl],
                        start=(c == 0), stop=(c == DC - 1),
                    )
                # rows 0:64 = k, 64:128 = v
                nc.scalar.mul(kT_aug[0:64, sl], ppA[0:64, :], -2.0)
                nc.scalar.copy(vT[64:128, sl], ppA[64:128, :])
                nc.scalar.square(sq[0:64, sl], ppA[0:64, :])
                # k2 sum for the PREVIOUS chunk (its square is long done,
                # so the PE does not stall on the ACT chain)
                if ss > 0:
                    emit_b_sum(ss - 1)
                for j in range(4):
                    t = 16 + ss * 4 + j
                    nc.sync.dma_start_transpose(
                        v_sb[:, t, 0:64], vT[64:128, t * 128 : (t + 1) * 128]
                    )
                # phase-1 tiles over the q-half key region
                for j in range(4):
                    t = ss * 4 + j
                    emit_d2_tile(t)
                    if t in early_dve:
                        early_po_q.append(t)
            emit_b_sum(3)

        # setup pools closed (pp/pb PSUM freed) -> open po
        po_pool = ctx.enter_context(tc.tile_pool(name="po", bufs=1, space="PSUM"))
        po = po_pool.tile([128, M], F32)

        # ------- rest of phase 1: tiles 16..31 (minus deferred), with po
        # groups for ready DVE tiles interleaved to fill PE slack -------
        pending_po = list(early_po_q)
        for t in range(16, KT):
            if t in defer_tiles:
                continue
            emit_d2_tile(t)
            if not _is_act_tile(t):
                pending_po.append(t)
            if len(pending_po) > 2:
                c = pending_po.pop(0)
                emit_po_tile(c, buf[:, c, :].bitcast(BF16))
        for c in pending_po:
            emit_po_tile(c, buf[:, c, :].bitcast(BF16))

        tc.no_sync_barrier()  # all Sqrt before all Exp: one table switch

        # ---------------- phase 2: exp (ACT tiles) + deferred DVE ----------------
        with ExitStack() as p2ctx:
            att_pool = p2ctx.enter_context(tc.tile_pool(name="att", bufs=3))
            # ACT tiles come in adjacent pairs -> one batched Exp per pair;
            # deferred DVE tiles interleave so the DVE works through phase 2
            pairs = [
                (act_tiles[i], act_tiles[i + 1])
                for i in range(0, len(act_tiles), 2)
            ]
            dq = list(defer_tiles)
            defer_counts = [2, 1, 1, 1, 2] if len(dq) == 7 else None
            for pi, (t0, t1) in enumerate(pairs):
                att2 = att_pool.tile([128, 2, M], BF16)
                nc.scalar.activation(
                    att2[:], buf[:, t0 : t0 + 2, :], AF.Exp, bias=ebias[:]
                )
                emit_po_tile(t0, att2[:, 0, :])
                emit_po_tile(t1, att2[:, 1, :])
                n_d = defer_counts[pi] if defer_counts else (1 if dq else 0)
                for _ in range(n_d):
                    if dq:
                        t = dq.pop(0)
                        emit_d2_tile(t)
                        emit_po_tile(t, buf[:, t, :].bitcast(BF16))
            while dq:
                t = dq.pop(0)
                emit_d2_tile(t)
                emit_po_tile(t, buf[:, t, :].bitcast(BF16))
            assert po_emitted[0] == PO_TOTAL

            # -------- finish: copy outT[0:65] to SBUF, DMA out --------
            # split across DVE and ACT to halve the tail
            oT_pool = p2ctx.enter_context(tc.tile_pool(name="oT", bufs=1))
            oT = oT_pool.tile([65, M], F32)
            nc.vector.tensor_copy(oT[:, 0 : M // 2], po[0:65, 0 : M // 2])
            nc.scalar.copy(oT[:, M // 2 : M], po[0:65, M // 2 : M])
            nc.sync.dma_start(out[:, 0 : M // 2], oT[:, 0 : M // 2])
            nc.sync.dma_start(out[:, M // 2 : M], oT[:, M // 2 : M])


_NC_CACHE = None


def build():
    global _NC_CACHE
    if _NC_CACHE is not None:
        return _NC_CACHE
    nc = bacc.Bacc("TRN2", target_bir_lowering=False, debug=False, num_devices=NCORES)
    xt_d = nc.declare_dram_parameter("xt", [DIN, S], BF16, isOutput=False)
    wt_d = nc.declare_dram_parameter("wt", [DIN, 3 * DOUT], BF16, isOutput=False)
    out_d = nc.declare_dram_parameter("out", [65, M], F32, isOutput=True)
    with tile.TileContext(nc) as tc:
        _body(tc, xt_d[:], wt_d[:], out_d[:])
    nc.compile()
    _NC_CACHE = nc
    return nc


def make_in_maps(x, Wq, Wk, Wv):
    bf16 = ml_dtypes.bfloat16
    wt = np.ascontiguousarray(
        np.concatenate(
            [np.asarray(W, np.float32).T for W in (Wq, Wk, Wv)], axis=1
        ).astype(bf16)
    )
    in_maps = []
    for c in range(NCORES):
        b, h = divmod(c, 2)
        xb = np.asarray(x[b], np.float32)
        xc = np.concatenate(
            [xb[h * M : (h + 1) * M], xb[(1 - h) * M : (2 - h) * M]], 0
        )
        in_maps.append({"xt": np.ascontiguousarray(xc.T.astype(bf16)), "wt": wt})
    return in_maps


def gather_out(results):
    out = np.zeros((B, S, DOUT), np.float32)
    for c in range(NCORES):
        b, h = divmod(c, 2)
        oT = np.asarray(results[c]["out"], np.float32)
        out[b, h * M : (h + 1) * M] = (oT[0:64] / oT[64:65]).T
    return out


def kernel(x, Wq, Wk, Wv):
    nc = build()
    in_maps = make_in_maps(x, Wq, Wk, Wv)
    res = run_bass_kernel_spmd(nc, in_maps, core_ids=list(range(NCORES)))
    return gather_out(res.results)
